# revision 5
# baseline (speedup 1.0000x reference)
"""GCNClusterNet Trainium2 kernel — 8-core SPMD.

Strategy (hardcoded for N=10000, NFEAT=1024, NHID=512, NOUT=128, K=50):
  - Row-shard x and adj across 8 cores (1250 rows each).
  - Cast adj shard to bf16 in DRAM once (SWDGE cast DMA), then feed both
    adj matmuls via hardware DMA-transpose loads (2-byte dtype only).
  - GCN matmuls in bf16 (fp32 PSUM accumulate); all-gather of xW1 / hW2
    activations (bf16) between stages.
  - k-means in fp32: per-iteration partial cluster stats + one packed
    [129, 50] AllReduce; softmax without max-subtraction (|logit| <= 30).
  - Outputs: mu (replicated), r / embeds / dist row-sharded, gathered on host.
"""
import numpy as np

import concourse.bass as bass
import concourse.mybir as mybir
import concourse.tile as tile
import concourse.bacc as bacc
from concourse.bass_utils import run_bass_kernel_spmd
from concourse.masks import make_identity

F32 = mybir.dt.float32
BF16 = mybir.dt.bfloat16
AF = mybir.ActivationFunctionType

NC = 8
N, NFEAT, NHID, NOUT, K = 10000, 1024, 512, 128, 50
TEMP = 30.0
P = 128
M_LOC = N // NC            # 1250 rows per core
NJT = (N + P - 1) // P     # 79 j-tiles (last has 16 valid rows)
JPAD = NJT * P             # 10112
# m-chunks for adj passes: DMA-transpose needs row count % 16 == 0
M_CHUNKS = [(0, 512), (512, 512), (1024, 240)]   # covers 1250 (+14 pad rows)
M_PADDED = 1264
NMB = (M_LOC + P - 1) // P  # 10 m-blocks of 128 (last 98 valid)


def _mb_real(mb):
    return min(P, M_LOC - mb * P)  # valid rows in m-block (98 for mb=9)


def build_kernel(num_iter: int):
    nc = bacc.Bacc("TRN2", target_bir_lowering=False, debug=False, num_devices=NC)

    x_d = nc.dram_tensor("x_sh", [M_LOC, NFEAT], F32, kind="ExternalInput").ap()
    adj_d = nc.dram_tensor("adj_sh", [M_LOC, N], F32, kind="ExternalInput").ap()
    W1_d = nc.dram_tensor("W1", [NFEAT, NHID], F32, kind="ExternalInput").ap()
    W2_d = nc.dram_tensor("W2", [NHID, NOUT], F32, kind="ExternalInput").ap()
    b1_d = nc.dram_tensor("b1", [NHID], F32, kind="ExternalInput").ap()
    b2_d = nc.dram_tensor("b2", [NOUT], F32, kind="ExternalInput").ap()
    mu0_d = nc.dram_tensor("init_mu", [K, NOUT], F32, kind="ExternalInput").ap()

    omu_d = nc.dram_tensor("out_mu", [K, NOUT], F32, kind="ExternalOutput").ap()
    or_d = nc.dram_tensor("out_r", [M_LOC, K], F32, kind="ExternalOutput").ap()
    oemb_d = nc.dram_tensor("out_embeds", [M_LOC, NOUT], F32, kind="ExternalOutput").ap()
    odist_d = nc.dram_tensor("out_dist", [M_LOC, K], F32, kind="ExternalOutput").ap()

    rg = [list(range(NC))]

    with tile.TileContext(nc) as tc:
        with (
            tc.tile_pool(name="persist", bufs=1) as pp,
            tc.tile_pool(name="km_state", bufs=2) as kmp,
            tc.tile_pool(name="dram", bufs=1, space="DRAM") as dram,
        ):
            # ---- DRAM scratch ----
            adjb = [dram.tile([rows if rows % 16 == 0 else rows, JPAD], BF16,
                              name=f"adjb{ci}")
                    for ci, (_, rows) in enumerate(M_CHUNKS)]
            xw1_agi = dram.tile([M_LOC, NHID], BF16, name="xw1_agi")
            xw1_ago = dram.tile([N, NHID], BF16, addr_space="Shared", name="xw1_ago")
            hw2_agi = dram.tile([M_LOC, NOUT], BF16, name="hw2_agi")
            hw2_ago = dram.tile([N, NOUT], BF16, addr_space="Shared", name="hw2_ago")

            # ---- persistent SBUF ----
            ones_row = pp.tile([1, P], F32)
            nc.vector.memset(ones_row[:], 1.0)
            ones_col = pp.tile([P, 1], F32)
            nc.vector.memset(ones_col[:], 1.0)
            ident = pp.tile([P, P], F32)
            make_identity(nc, ident[:])
            eps_col = pp.tile([P, 1], F32)
            nc.vector.memset(eps_col[:], 1e-30)
            b1_bc = pp.tile([P, NHID], F32)     # b1 broadcast across partitions
            b2_col = pp.tile([P, 1], F32)       # b2 as per-partition column
            hT_sb = pp.tile([P, NHID // P, M_PADDED], BF16)      # [c, ct, m]
            hw2_sb = pp.tile([P, NJT, NOUT], BF16)               # [j, jt, o]
            dataT_sb = pp.tile([P, M_PADDED], F32)               # [d, m]
            data_sb = pp.tile([P, NMB, P], F32)                  # [m, mb, d]

            # =========== Phase A: adj cast, weights, x@W1, AG ===========
            with (
                tc.tile_pool(name="pa_sbuf", bufs=2) as pa,
                tc.tile_pool(name="pa_w", bufs=1) as paw,
                tc.tile_pool(name="pa_psum", bufs=2, space="PSUM") as pa_ps,
                tc.tile_pool(name="pa_stage", bufs=2, space="PSUM") as pa_st,
            ):
                # adj f32 -> bf16 DRAM copies (chunked so phase B can pipeline)
                for ci, (r0, rows) in enumerate(M_CHUNKS):
                    rreal = min(rows, M_LOC - r0)
                    nc.gpsimd.dma_start(adjb[ci][0:rreal, 0:N], adj_d[r0:r0 + rreal, :])
                if True:  # zero the 14 pad rows of the last chunk
                    zpad = pa.tile([16, JPAD], BF16, name="zpad")
                    nc.vector.memset(zpad[:], 0.0)
                    nc.sync.dma_start(adjb[2][226:240, :], zpad[0:14, :])

                # weights / biases
                W1_bf = paw.tile([P, NFEAT // P, NHID], BF16)
                nc.gpsimd.dma_start(
                    W1_bf[:], W1_d.rearrange("(t p) c -> p t c", p=P))
                W2_bf = pp.tile([P, NHID // P, NOUT], BF16)
                nc.gpsimd.dma_start(
                    W2_bf[:], W2_d.rearrange("(t p) c -> p t c", p=P))
                b1_row = pa.tile([1, NHID], F32, name="b1_row")
                nc.sync.dma_start(b1_row[:], b1_d.rearrange("(a c) -> a c", a=1))
                bb_ps = pa_ps.tile([P, NHID], F32, name="bb_ps")
                nc.tensor.matmul(bb_ps[:], ones_row[:], b1_row[:], start=True, stop=True)
                nc.vector.tensor_copy(b1_bc[:], bb_ps[:])
                nc.sync.dma_start(b2_col[:], b2_d.rearrange("(p a) -> p a", a=1))

                # x shard -> xT (bf16) via PE transpose
                xT_sb = paw.tile([P, NFEAT // P, M_PADDED], BF16)  # [f, ft, m]
                for mb in range(NMB):
                    realm = _mb_real(mb)
                    mw = min(P, M_PADDED - mb * P)
                    x_t = pa.tile([P, NFEAT], F32, name="x_t")
                    nc.sync.dma_start(x_t[0:realm, :], x_d[mb * P:mb * P + realm, :])
                    for ft in range(NFEAT // P):
                        st = pa_st.tile([P, P], F32, name="xt_st")
                        nc.tensor.transpose(st[:], x_t[:, ft * P:(ft + 1) * P], ident[:])
                        nc.vector.tensor_copy(
                            xT_sb[:, ft, mb * P:mb * P + mw], st[:, 0:mw])

                # xW1 = x @ W1 (bf16), write AG input
                for mb in range(NMB):
                    realm = _mb_real(mb)
                    mw = min(P, M_PADDED - mb * P)
                    acc = pa_ps.tile([P, NHID], F32, name="xw1_acc")
                    for ft in range(NFEAT // P):
                        nc.tensor.matmul(
                            acc[0:mw, :],
                            xT_sb[:, ft, mb * P:mb * P + mw],
                            W1_bf[:, ft, :],
                            start=(ft == 0), stop=(ft == NFEAT // P - 1))
                    xw1_bf = pa.tile([P, NHID], BF16, name="xw1_bf")
                    nc.vector.tensor_copy(xw1_bf[0:realm, :], acc[0:realm, :])
                    nc.sync.dma_start(
                        xw1_agi[mb * P:mb * P + realm, :], xw1_bf[0:realm, :])

                nc.gpsimd.collective_compute(
                    "AllGather", mybir.AluOpType.bypass,
                    ins=[xw1_agi.opt()], outs=[xw1_ago.opt()], replica_groups=rg)

            # =========== Phase B: h = relu(adj @ xW1 + b1); hT ===========
            with (
                tc.tile_pool(name="pb_xw1", bufs=1) as pbx,
                tc.tile_pool(name="pb_sbuf", bufs=3) as pb,
                tc.tile_pool(name="pb_eps", bufs=2) as pbe,
                tc.tile_pool(name="pb_acc", bufs=1, space="PSUM") as pb_acc,
                tc.tile_pool(name="pb_stage", bufs=2, space="PSUM") as pb_st,
            ):
                xw1_sb = pbx.tile([P, NJT, NHID], BF16)  # [j, jt, c]
                nfull = (N // P) * P  # 9984
                nc.sync.dma_start(
                    xw1_sb[:, 0:N // P, :],
                    xw1_ago[0:nfull, :].rearrange("(t p) c -> p t c", p=P))
                nc.sync.dma_start(xw1_sb[0:N - nfull, N // P, :], xw1_ago[nfull:N, :])

                for ci, (r0, rows) in enumerate(M_CHUNKS):
                    nblk = (rows + P - 1) // P
                    accs = []
                    for bi in range(nblk):
                        bw = min(P, rows - bi * P)
                        accs.append((pb_acc.tile([P, NHID], F32, name=f"h_acc{bi}"), bw))
                    for jt in range(NJT):
                        kj = min(P, N - jt * P)
                        adjT = pb.tile([P, 512], BF16, name="adjT_b")
                        nc.sync.dma_start_transpose(
                            adjT[:, 0:rows], adjb[ci][:, jt * P:(jt + 1) * P])
                        for bi in range(nblk):
                            acc, bw = accs[bi]
                            nc.tensor.matmul(
                                acc[0:bw, :],
                                adjT[0:kj, bi * P:bi * P + bw],
                                xw1_sb[0:kj, jt, :],
                                start=(jt == 0), stop=(jt == NJT - 1))
                    for bi in range(nblk):
                        acc, bw = accs[bi]
                        mb = (r0 + bi * P) // P
                        realm = max(0, min(M_LOC - (r0 + bi * P), P))
                        h_tmp = pbe.tile([P, NHID], F32, name="h_tmp")
                        nc.vector.tensor_tensor(
                            out=h_tmp[0:bw, :], in0=acc[0:bw, :], in1=b1_bc[0:bw, :],
                            op=mybir.AluOpType.add)
                        h_rl = pbe.tile([P, NHID], F32, name="h_rl")
                        nc.scalar.activation(h_rl[0:bw, :], h_tmp[0:bw, :], AF.Relu)
                        for ct in range(NHID // P):
                            st = pb_st.tile([P, P], F32, name="hT_st")
                            nc.tensor.transpose(
                                st[:, 0:bw], h_rl[0:bw, ct * P:(ct + 1) * P],
                                ident[0:bw, 0:bw])
                            nc.vector.tensor_copy(
                                hT_sb[:, ct, mb * P:mb * P + bw], st[:, 0:bw])

            # =========== Phase B2: hW2 = h @ W2 (bf16) + AG ===========
            with (
                tc.tile_pool(name="pc_sbuf", bufs=2) as pc,
                tc.tile_pool(name="pc_psum", bufs=2, space="PSUM") as pc_ps,
            ):
                for mb in range(NMB):
                    realm = _mb_real(mb)
                    mw = min(P, M_PADDED - mb * P)
                    acc = pc_ps.tile([P, NOUT], F32, name="hw2_acc")
                    for ct in range(NHID // P):
                        nc.tensor.matmul(
                            acc[0:mw, :],
                            hT_sb[:, ct, mb * P:mb * P + mw],
                            W2_bf[:, ct, :],
                            start=(ct == 0), stop=(ct == NHID // P - 1))
                    hw2_bf = pc.tile([P, NOUT], BF16, name="hw2_bf")
                    nc.vector.tensor_copy(hw2_bf[0:realm, :], acc[0:realm, :])
                    nc.sync.dma_start(
                        hw2_agi[mb * P:mb * P + realm, :], hw2_bf[0:realm, :])
                nc.gpsimd.collective_compute(
                    "AllGather", mybir.AluOpType.bypass,
                    ins=[hw2_agi.opt()], outs=[hw2_ago.opt()], replica_groups=rg)
                nfull = (N // P) * P
                nc.sync.dma_start(
                    hw2_sb[:, 0:N // P, :],
                    hw2_ago[0:nfull, :].rearrange("(t p) c -> p t c", p=P))
                nc.sync.dma_start(hw2_sb[0:N - nfull, N // P, :], hw2_ago[nfull:N, :])

            # ==== Phase C: embedsT = (adj @ hW2 + b2)^T; normalize -> data ====
            with (
                tc.tile_pool(name="pd_sbuf", bufs=3) as pd,
                tc.tile_pool(name="pd_eps", bufs=2) as pde,
                tc.tile_pool(name="pd_acc", bufs=2, space="PSUM") as pd_acc,
                tc.tile_pool(name="pd_stage", bufs=2, space="PSUM") as pd_st,
            ):
                eT_sb = pd.tile([P, M_PADDED], F32, bufs=1, name="eT_sb")  # [o, m]
                for ci, (r0, rows) in enumerate(M_CHUNKS):
                    acc = pd_acc.tile([P, 512], F32, name="eT_acc")
                    for jt in range(NJT):
                        kj = min(P, N - jt * P)
                        adjT = pd.tile([P, 512], BF16, name="adjT_c")
                        nc.sync.dma_start_transpose(
                            adjT[:, 0:rows], adjb[ci][:, jt * P:(jt + 1) * P])
                        nc.tensor.matmul(
                            acc[:, 0:rows],
                            hw2_sb[0:kj, jt, :],
                            adjT[0:kj, 0:rows],
                            start=(jt == 0), stop=(jt == NJT - 1))
                    # + b2 (per-partition scalar) -> eT_sb
                    nc.vector.tensor_scalar(
                        out=eT_sb[:, r0:r0 + rows], in0=acc[:, 0:rows],
                        scalar1=b2_col[:], scalar2=None, op0=mybir.AluOpType.add)

                # per m-block: embeds natural, norms, data, dataT
                for mb in range(NMB):
                    realm = _mb_real(mb)
                    mw = min(P, M_PADDED - mb * P)
                    st = pd_st.tile([P, P], F32, name="e_st")
                    nc.tensor.transpose(
                        st[0:mw, :], eT_sb[:, mb * P:mb * P + mw], ident[:])
                    emb = pde.tile([P, NOUT], F32, name="emb")
                    nc.vector.tensor_copy(emb[0:mw, :], st[0:mw, :])
                    nc.sync.dma_start(
                        oemb_d[mb * P:mb * P + realm, :], emb[0:realm, :])
                    sq = pde.tile([P, NOUT], F32, name="sq")
                    n2 = pde.tile([P, 1], F32, name="n2")
                    nc.scalar.activation(
                        sq[0:mw, :], emb[0:mw, :], AF.Square, accum_out=n2[0:mw, :])
                    nrm = pde.tile([P, 1], F32, name="nrm")
                    nc.scalar.activation(
                        nrm[0:mw, :], n2[0:mw, :], AF.Sqrt, bias=eps_col[0:mw, :])
                    rnrm = pde.tile([P, 1], F32, name="rnrm")
                    nc.vector.reciprocal(rnrm[0:mw, :], nrm[0:mw, :])
                    nc.vector.tensor_scalar(
                        out=data_sb[0:mw, mb, :], in0=emb[0:mw, :],
                        scalar1=rnrm[0:mw, :], scalar2=None, op0=mybir.AluOpType.mult)
                    st2 = pd_st.tile([P, P], F32, name="d_st")
                    nc.tensor.transpose(
                        st2[:, 0:mw], data_sb[0:mw, mb, :], ident[0:mw, 0:mw])
                    nc.vector.tensor_copy(dataT_sb[:, mb * P:mb * P + mw], st2[:, 0:mw])

            # =========== Phase D: k-means ===========
            with (
                tc.tile_pool(name="pk_sbuf", bufs=3) as pk,
                tc.tile_pool(name="pk_r", bufs=12) as pkr,
                tc.tile_pool(name="pk_dist", bufs=2, space="PSUM") as pk_dist,
                tc.tile_pool(name="pk_stat", bufs=1, space="PSUM") as pk_stat,
                tc.tile_pool(name="pk_misc", bufs=1, space="PSUM") as pk_misc,
            ):
                muT = kmp.tile([P, K], F32, name="muT0")
                nc.sync.dma_start(muT[:], mu0_d.rearrange("k d -> d k"))

                def dist_pass(muT_cur, store_out):
                    r_tiles = []
                    for mb in range(NMB):
                        realm = _mb_real(mb)
                        mw = min(P, M_PADDED - mb * P)
                        dps = pk_dist.tile([P, K], F32, name="dist_ps")
                        nc.tensor.matmul(
                            dps[0:mw, :], dataT_sb[:, mb * P:mb * P + mw],
                            muT_cur[:], start=True, stop=True)
                        rt = pkr.tile([P, K], F32, name="r_t")
                        se = pk.tile([P, 1], F32, name="se")
                        nc.scalar.activation(
                            rt[0:mw, :], dps[0:mw, :], AF.Exp, scale=TEMP,
                            accum_out=se[0:mw, :])
                        rse = pk.tile([P, 1], F32, name="rse")
                        nc.vector.reciprocal(rse[0:mw, :], se[0:mw, :])
                        nc.vector.tensor_scalar(
                            out=rt[0:mw, :], in0=rt[0:mw, :], scalar1=rse[0:mw, :],
                            scalar2=None, op0=mybir.AluOpType.mult)
                        if store_out:
                            dst = pk.tile([P, K], F32, name="dist_sb")
                            nc.vector.tensor_copy(dst[0:realm, :], dps[0:realm, :])
                            nc.sync.dma_start(
                                odist_d[mb * P:mb * P + realm, :], dst[0:realm, :])
                            nc.sync.dma_start(
                                or_d[mb * P:mb * P + realm, :], rt[0:realm, :])
                        r_tiles.append((rt, realm))
                    return r_tiles

                for it in range(num_iter + 1):
                    st_ari = dram.tile([P + 1, K], F32, name=f"st_ari{it}")
                    st_aro = dram.tile([P + 1, K], F32, addr_space="Shared",
                                       name=f"st_aro{it}")
                    r_tiles = dist_pass(muT, store_out=False)
                    cm_ps = pk_stat.tile([P, K], F32, name="cm_ps")
                    cr_ps = pk_stat.tile([1, K], F32, name="cr_ps")
                    for mb in range(NMB):
                        rt, realm = r_tiles[mb]
                        nc.tensor.matmul(
                            cm_ps[:], data_sb[0:realm, mb, :], rt[0:realm, :],
                            start=(mb == 0), stop=(mb == NMB - 1))
                        nc.tensor.matmul(
                            cr_ps[:], ones_col[0:realm, :], rt[0:realm, :],
                            start=(mb == 0), stop=(mb == NMB - 1))
                    cm_sb = pk.tile([P, K], F32, name="cm_sb")
                    cr_sb = pk.tile([1, K], F32, name="cr_sb")
                    nc.vector.tensor_copy(cm_sb[:], cm_ps[:])
                    nc.vector.tensor_copy(cr_sb[:], cr_ps[:])
                    nc.sync.dma_start(st_ari[0:P, :], cm_sb[:])
                    nc.sync.dma_start(st_ari[P:P + 1, :], cr_sb[:])
                    nc.gpsimd.collective_compute(
                        "AllReduce", mybir.AluOpType.add,
                        ins=[st_ari.opt()], outs=[st_aro.opt()], replica_groups=rg)
                    cm2 = pk.tile([P, K], F32, name="cm2")
                    cr2 = pk.tile([1, K], F32, name="cr2")
                    nc.sync.dma_start(cm2[:], st_aro[0:P, :])
                    nc.sync.dma_start(cr2[:], st_aro[P:P + 1, :])
                    cri = pk.tile([1, K], F32, name="cri")
                    nc.vector.reciprocal(cri[:], cr2[:])
                    bc_ps = pk_misc.tile([P, K], F32, name="bc_ps")
                    nc.tensor.matmul(bc_ps[:], ones_row[:], cri[:], start=True, stop=True)
                    muT = kmp.tile([P, K], F32, name="muT")
                    nc.vector.tensor_tensor(
                        out=muT[:], in0=cm2[:], in1=bc_ps[:], op=mybir.AluOpType.mult)

                # final pass: dist, r outputs with final mu
                dist_pass(muT, store_out=True)

                # mu output: transpose muT -> [K, NOUT]
                mu_st = pk_misc.tile([K, P], F32, name="mu_st")
                nc.tensor.transpose(mu_st[:], muT[:, 0:K], ident[:])
                mu_sb = pk.tile([K, P], F32, name="mu_sb")
                nc.vector.tensor_copy(mu_sb[:], mu_st[:])
                nc.sync.dma_start(omu_d, mu_sb[:])

    nc.compile()
    return nc


_NC_CACHE = {}


def get_nc(num_iter: int):
    if num_iter not in _NC_CACHE:
        _NC_CACHE[num_iter] = build_kernel(num_iter)
    return _NC_CACHE[num_iter]


def kernel(x, adj, W1, b1, W2, b2, init_mu, num_iter):
    x = np.ascontiguousarray(np.asarray(x, dtype=np.float32))
    adj = np.ascontiguousarray(np.asarray(adj, dtype=np.float32))
    W1 = np.ascontiguousarray(np.asarray(W1, dtype=np.float32))
    W2 = np.ascontiguousarray(np.asarray(W2, dtype=np.float32))
    b1 = np.ascontiguousarray(np.asarray(b1, dtype=np.float32))
    b2 = np.ascontiguousarray(np.asarray(b2, dtype=np.float32))
    init_mu = np.ascontiguousarray(np.asarray(init_mu, dtype=np.float32))
    n_it = int(np.asarray(num_iter))

    nc = get_nc(n_it)
    in_maps = []
    for c in range(NC):
        sl = slice(c * M_LOC, (c + 1) * M_LOC)
        in_maps.append({
            "x_sh": x[sl], "adj_sh": adj[sl],
            "W1": W1, "W2": W2, "b1": b1, "b2": b2, "init_mu": init_mu,
        })
    res = run_bass_kernel_spmd(nc, in_maps, core_ids=list(range(NC)))
    outs = res.results
    mu = outs[0]["out_mu"]
    r = np.concatenate([outs[c]["out_r"] for c in range(NC)], axis=0)
    embeds = np.concatenate([outs[c]["out_embeds"] for c in range(NC)], axis=0)
    dist = np.concatenate([outs[c]["out_dist"] for c in range(NC)], axis=0)
    return (mu, r, embeds, dist)


# revision 11
# speedup vs baseline: 1.0030x; 1.0030x over previous
"""GCNClusterNet Trainium2 kernel — 8-core SPMD.

Strategy (hardcoded for N=10000, NFEAT=1024, NHID=512, NOUT=128, K=50):
  - Row-shard x and adj across 8 cores (1250 rows each).
  - Cast adj and x shards to bf16 in DRAM once (SWDGE cast DMA), then feed
    all transposed operands via hardware DMA-transpose loads (few, large).
  - GCN matmuls in bf16 (fp32 PSUM accumulate); bf16 all-gather of xW1 /
    hW2 activations between stages. h kept transposed (hT) end-to-end:
    mm1 emits hT directly so bias+relu+bf16-cast fuse into one ACT op.
  - k-means in fp32: per-iteration partial cluster stats packed [128, 51]
    (cluster_r in column 50) + one AllReduce; softmax without max-sub.
  - Outputs: mu (replicated), r / embeds / dist row-sharded, gathered on host.
"""
import numpy as np

import concourse.bass as bass
import concourse.mybir as mybir
import concourse.tile as tile
import concourse.bacc as bacc
from concourse.bass_utils import run_bass_kernel_spmd
from concourse.masks import make_identity

F32 = mybir.dt.float32
BF16 = mybir.dt.bfloat16
AF = mybir.ActivationFunctionType

NC = 8
N, NFEAT, NHID, NOUT, K = 10000, 1024, 512, 128, 50
TEMP = 30.0
P = 128
M_LOC = N // NC            # 1250 rows per core
NJT = (N + P - 1) // P     # 79 j-tiles (last has 16 valid rows)
JPAD = NJT * P             # 10112
M_PADDED = 1264            # 1250 padded to mult of 16
NMB = (M_LOC + P - 1) // P  # 10 m-blocks of 128 (last 98 valid)
NCT = NHID // P            # 4 c-tiles
NFT = NFEAT // P           # 8 f-tiles
# phase B psum groups: (row offset, m-chunk widths); 4 c-tiles x chunks <= 8 banks
B_GROUPS = [(0, [512, 512]), (1024, [240])]
C_CHUNKS = [512, 512, 240]


def _mb_real(mb):
    return min(P, M_LOC - mb * P)  # valid rows in m-block (98 for mb=9)


def _collective(nc, kind, op, in_ap, out_ap, rg, model, shard_rows=None):
    if not model:
        nc.gpsimd.collective_compute(kind, op, ins=[in_ap.opt()], outs=[out_ap.opt()],
                                     replica_groups=rg)
    elif kind == "AllGather":
        for c in range(NC):
            nc.sync.dma_start(out_ap[c * shard_rows:(c + 1) * shard_rows, :], in_ap[:])
    else:
        nc.sync.dma_start(out_ap[:], in_ap[:])


def build_kernel(num_iter: int, phases: str = "ABCD", model_single: bool = False):
    nc = bacc.Bacc("TRN2", target_bir_lowering=False, debug=False,
                   num_devices=(1 if model_single else NC))

    x_d = nc.dram_tensor("x_sh", [M_LOC, NFEAT], F32, kind="ExternalInput").ap()
    adj_d = nc.dram_tensor("adj_sh", [M_LOC, N], F32, kind="ExternalInput").ap()
    W1_d = nc.dram_tensor("W1", [NFEAT, NHID], F32, kind="ExternalInput").ap()
    W2_d = nc.dram_tensor("W2", [NHID, NOUT], F32, kind="ExternalInput").ap()
    b1_d = nc.dram_tensor("b1", [NHID], F32, kind="ExternalInput").ap()
    b2_d = nc.dram_tensor("b2", [NOUT], F32, kind="ExternalInput").ap()
    mu0_d = nc.dram_tensor("init_mu", [K, NOUT], F32, kind="ExternalInput").ap()

    omu_d = nc.dram_tensor("out_mu", [K, NOUT], F32, kind="ExternalOutput").ap()
    or_d = nc.dram_tensor("out_r", [M_LOC, K], F32, kind="ExternalOutput").ap()
    oemb_d = nc.dram_tensor("out_embeds", [M_LOC, NOUT], F32, kind="ExternalOutput").ap()
    odist_d = nc.dram_tensor("out_dist", [M_LOC, K], F32, kind="ExternalOutput").ap()

    rg = [list(range(NC))]
    shr = "Local" if model_single else "Shared"
    NFB = (N // P) * P  # 9984 full-tile rows of gathered activations

    with tile.TileContext(nc) as tc:
        with (
            tc.tile_pool(name="persist", bufs=1) as pp,
            tc.tile_pool(name="km_state", bufs=2) as kmp,
            tc.tile_pool(name="dram", bufs=1, space="DRAM") as dram,
        ):
            # ---- DRAM scratch ----
            adjb = dram.tile([M_PADDED, JPAD], BF16, name="adjb")
            xb = dram.tile([M_PADDED, NFEAT], BF16, name="xb")
            xw1_agi = dram.tile([M_LOC, NHID], BF16, name="xw1_agi")
            xw1_ago = dram.tile([N, NHID], BF16, addr_space=shr, name="xw1_ago")
            hw2_agi = dram.tile([M_LOC, NOUT], BF16, name="hw2_agi")
            hw2_ago = dram.tile([N, NOUT], BF16, addr_space=shr, name="hw2_ago")

            # ---- persistent SBUF ----
            ones_row = pp.tile([1, P], F32)
            nc.vector.memset(ones_row[:], 1.0)
            ones_col = pp.tile([P, 1], F32)
            nc.vector.memset(ones_col[:], 1.0)
            ident = pp.tile([P, P], F32)
            make_identity(nc, ident[:])
            eps_col = pp.tile([P, 1], F32)
            nc.vector.memset(eps_col[:], 1e-30)
            b1_cols = pp.tile([P, NCT], F32)      # b1 as per-partition columns
            b2_col = pp.tile([P, 1], F32)
            W2_bf = pp.tile([P, NCT, NOUT], BF16)
            hT_sb = pp.tile([P, NCT, M_PADDED], BF16)       # [c, ct, m]
            hw2_sb = pp.tile([P, NJT, NOUT], BF16)          # [j, jt, o]
            dataT_sb = pp.tile([P, M_PADDED], F32)          # [d, m]
            data_sb = pp.tile([P, NMB, P], F32)             # [m, mb, d]
            emb_all = pp.tile([P, NMB, NOUT], F32)          # [m, mb, o]

            # =========== Phase A: casts, weights, x@W1, AG ===========
            if "A" in phases:
             with (
                tc.tile_pool(name="pa_sbuf", bufs=2) as pa,
                tc.tile_pool(name="pa_w", bufs=1) as paw,
                tc.tile_pool(name="pa_psum", bufs=2, space="PSUM") as pa_ps,
             ):
                # adj / x f32 -> bf16 DRAM (SWDGE cast); zero adj pad rows
                nc.gpsimd.dma_start(adjb[0:1024, 0:N], adj_d[0:1024, :])
                nc.gpsimd.dma_start(adjb[1024:M_LOC, 0:N], adj_d[1024:M_LOC, :])
                zpad = pa.tile([16, JPAD], BF16, name="zpad")
                nc.vector.memset(zpad[:], 0.0)
                nc.sync.dma_start(adjb[M_LOC:M_PADDED, :], zpad[0:M_PADDED - M_LOC, :])
                nc.gpsimd.dma_start(xb[0:M_LOC, :], x_d)

                # weights / biases
                W1_bf = paw.tile([P, NFT, NHID], BF16)
                nc.gpsimd.dma_start(W1_bf[:], W1_d.rearrange("(t p) c -> p t c", p=P))
                nc.gpsimd.dma_start(W2_bf[:], W2_d.rearrange("(t p) c -> p t c", p=P))
                nc.sync.dma_start(b1_cols[:], b1_d.rearrange("(t p) -> p t", p=P))
                nc.sync.dma_start(b2_col[:], b2_d.rearrange("(p a) -> p a", a=1))

                # xT via DMA-transpose: 8 loads of [M_PADDED, 128] -> [128, M_PADDED]
                xT_sb = paw.tile([P, NFT, M_PADDED], BF16)
                for ft in range(NFT):
                    nc.sync.dma_start_transpose(
                        xT_sb[:, ft, :], xb[:, ft * P:(ft + 1) * P])

                # xW1 = x @ W1 (bf16, natural out), write AG input
                for mb in range(NMB):
                    realm = _mb_real(mb)
                    mw = min(P, M_PADDED - mb * P)
                    acc = pa_ps.tile([P, NHID], F32, name="xw1_acc")
                    for ft in range(NFT):
                        nc.tensor.matmul(
                            acc[0:mw, :],
                            xT_sb[:, ft, mb * P:mb * P + mw],
                            W1_bf[:, ft, :],
                            start=(ft == 0), stop=(ft == NFT - 1))
                    xw1_bf = pa.tile([P, NHID], BF16, name="xw1_bf")
                    nc.vector.tensor_copy(xw1_bf[0:realm, :], acc[0:realm, :])
                    nc.sync.dma_start(
                        xw1_agi[mb * P:mb * P + realm, :], xw1_bf[0:realm, :])

                _collective(nc, "AllGather", mybir.AluOpType.bypass,
                            xw1_agi, xw1_ago, rg, model_single, M_LOC)

            # ====== Phase B: hT = relu(adj @ xW1 + b1)^T  (direct hT out) ======
            if "B" in phases:
             with (
                tc.tile_pool(name="pb_xw1", bufs=1) as pbx,
                tc.tile_pool(name="pb_sbuf", bufs=3) as pb,
                tc.tile_pool(name="pb_acc", bufs=1, space="PSUM") as pb_acc,
             ):
                xw1_sb = pbx.tile([P, NJT, NHID], BF16)  # [j, jt, c]
                nc.sync.dma_start(
                    xw1_sb[:, 0:N // P, :],
                    xw1_ago[0:NFB, :].rearrange("(t p) c -> p t c", p=P))
                nc.sync.dma_start(xw1_sb[0:N - NFB, N // P, :], xw1_ago[NFB:N, :])

                for gi, (g0, chunks) in enumerate(B_GROUPS):
                    grows = sum(chunks)
                    accs = {}
                    for ct in range(NCT):
                        for mi, cw in enumerate(chunks):
                            accs[(ct, mi)] = pb_acc.tile(
                                [P, cw], F32, name=f"hT_acc{ct}_{mi}")
                    for jt in range(NJT):
                        kj = min(P, N - jt * P)
                        adjT = pb.tile([P, 1024], BF16, name="adjT_b")
                        nc.sync.dma_start_transpose(
                            adjT[:, 0:grows],
                            adjb[g0:g0 + grows, jt * P:(jt + 1) * P])
                        for ct in range(NCT):
                            for mi, cw in enumerate(chunks):
                                moff = sum(chunks[:mi])
                                nc.tensor.matmul(
                                    accs[(ct, mi)][:],
                                    xw1_sb[0:kj, jt, ct * P:(ct + 1) * P],
                                    adjT[0:kj, moff:moff + cw],
                                    start=(jt == 0), stop=(jt == NJT - 1))
                    for ct in range(NCT):
                        for mi, cw in enumerate(chunks):
                            moff = g0 + sum(chunks[:mi])
                            nc.scalar.activation(
                                hT_sb[:, ct, moff:moff + cw], accs[(ct, mi)][:],
                                AF.Relu, bias=b1_cols[:, ct:ct + 1])

            # =========== Phase B2: hW2 = h @ W2 (bf16) + AG ===========
            if "B" in phases and "C" in phases:
             with (
                tc.tile_pool(name="pc_sbuf", bufs=2) as pc,
                tc.tile_pool(name="pc_psum", bufs=2, space="PSUM") as pc_ps,
             ):
                for mb in range(NMB):
                    realm = _mb_real(mb)
                    mw = min(P, M_PADDED - mb * P)
                    acc = pc_ps.tile([P, NOUT], F32, name="hw2_acc")
                    for ct in range(NCT):
                        nc.tensor.matmul(
                            acc[0:mw, :],
                            hT_sb[:, ct, mb * P:mb * P + mw],
                            W2_bf[:, ct, :],
                            start=(ct == 0), stop=(ct == NCT - 1))
                    hw2_bf = pc.tile([P, NOUT], BF16, name="hw2_bf")
                    nc.vector.tensor_copy(hw2_bf[0:realm, :], acc[0:realm, :])
                    nc.sync.dma_start(
                        hw2_agi[mb * P:mb * P + realm, :], hw2_bf[0:realm, :])
                _collective(nc, "AllGather", mybir.AluOpType.bypass,
                            hw2_agi, hw2_ago, rg, model_single, M_LOC)
                nc.sync.dma_start(
                    hw2_sb[:, 0:N // P, :],
                    hw2_ago[0:NFB, :].rearrange("(t p) c -> p t c", p=P))
                nc.sync.dma_start(hw2_sb[0:N - NFB, N // P, :], hw2_ago[NFB:N, :])

            # ==== Phase C: embedsT = (adj @ hW2 + b2)^T (j-outer); data ====
            if "C" in phases:
             with (
                tc.tile_pool(name="pd_sbuf", bufs=3) as pd,
                tc.tile_pool(name="pd_eps", bufs=2) as pde,
                tc.tile_pool(name="pd_acc", bufs=1, space="PSUM") as pd_acc,
                tc.tile_pool(name="pd_stage", bufs=2, space="PSUM") as pd_st,
             ):
                eT_sb = pd.tile([P, M_PADDED], F32, bufs=1, name="eT_sb")  # [o, m]
                eaccs = [pd_acc.tile([P, cw], F32, name=f"eT_acc{mi}")
                         for mi, cw in enumerate(C_CHUNKS)]
                for jt in range(NJT):
                    kj = min(P, N - jt * P)
                    adjT = pd.tile([P, M_PADDED], BF16, name="adjT_c")
                    nc.sync.dma_start_transpose(
                        adjT[:], adjb[:, jt * P:(jt + 1) * P])
                    for mi, cw in enumerate(C_CHUNKS):
                        moff = sum(C_CHUNKS[:mi])
                        nc.tensor.matmul(
                            eaccs[mi][:],
                            hw2_sb[0:kj, jt, :],
                            adjT[0:kj, moff:moff + cw],
                            start=(jt == 0), stop=(jt == NJT - 1))
                for mi, cw in enumerate(C_CHUNKS):
                    moff = sum(C_CHUNKS[:mi])
                    nc.vector.tensor_scalar(
                        out=eT_sb[:, moff:moff + cw], in0=eaccs[mi][:],
                        scalar1=b2_col[:], scalar2=None, op0=mybir.AluOpType.add)

                # per m-block: embeds natural, norms, data, dataT
                for mb in range(NMB):
                    realm = _mb_real(mb)
                    mw = min(P, M_PADDED - mb * P)
                    st = pd_st.tile([P, P], F32, name="e_st")
                    nc.tensor.transpose(
                        st[0:mw, :], eT_sb[:, mb * P:mb * P + mw], ident[:])
                    nc.vector.tensor_copy(emb_all[0:mw, mb, :], st[0:mw, :])
                    sq = pde.tile([P, NOUT], F32, name="sq")
                    n2 = pde.tile([P, 1], F32, name="n2")
                    nc.scalar.activation(
                        sq[0:mw, :], emb_all[0:mw, mb, :], AF.Square,
                        accum_out=n2[0:mw, :])
                    nrm = pde.tile([P, 1], F32, name="nrm")
                    nc.scalar.activation(
                        nrm[0:mw, :], n2[0:mw, :], AF.Sqrt, bias=eps_col[0:mw, :])
                    rnrm = pde.tile([P, 1], F32, name="rnrm")
                    nc.vector.reciprocal(rnrm[0:mw, :], nrm[0:mw, :])
                    nc.vector.tensor_scalar(
                        out=data_sb[0:mw, mb, :], in0=emb_all[0:mw, mb, :],
                        scalar1=rnrm[0:mw, :], scalar2=None, op0=mybir.AluOpType.mult)
                    st2 = pd_st.tile([P, P], F32, name="d_st")
                    nc.tensor.transpose(
                        st2[:, 0:mw], data_sb[0:mw, mb, :], ident[0:mw, 0:mw])
                    nc.vector.tensor_copy(dataT_sb[:, mb * P:mb * P + mw], st2[:, 0:mw])
                # embeds out (batched: 9 full tiles + 98-row tail)
                nc.sync.dma_start(
                    oemb_d[0:(NMB - 1) * P, :].rearrange("(t p) c -> p t c", p=P),
                    emb_all[:, 0:NMB - 1, :])
                nc.sync.dma_start(oemb_d[(NMB - 1) * P:M_LOC, :],
                                  emb_all[0:_mb_real(NMB - 1), NMB - 1, :])

            # =========== Phase D: k-means ===========
            if "D" in phases:
             with (
                tc.tile_pool(name="pk_sbuf", bufs=2) as pk,
                tc.tile_pool(name="pk_r", bufs=12) as pkr,
                tc.tile_pool(name="pk_dist", bufs=2, space="PSUM") as pk_dist,
                tc.tile_pool(name="pk_stat", bufs=1, space="PSUM") as pk_stat,
                tc.tile_pool(name="pk_misc", bufs=1, space="PSUM") as pk_misc,
             ):
                muT = kmp.tile([P, K], F32, name="muT0")
                nc.sync.dma_start(muT[:], mu0_d.rearrange("k d -> d k"))

                r_all = pp.tile([P, NMB, K], F32)
                dist_all = pp.tile([P, NMB, K], F32)

                def dist_pass(muT_cur, store_out):
                    r_tiles = []
                    for mb in range(NMB):
                        realm = _mb_real(mb)
                        mw = min(P, M_PADDED - mb * P)
                        dps = pk_dist.tile([P, K], F32, name="dist_ps")
                        nc.tensor.matmul(
                            dps[0:mw, :], dataT_sb[:, mb * P:mb * P + mw],
                            muT_cur[:], start=True, stop=True)
                        rt = pkr.tile([P, K], F32, name="r_t")
                        se = pkr.tile([P, 1], F32, name="se")
                        nc.scalar.activation(
                            rt[0:mw, :], dps[0:mw, :], AF.Exp, scale=TEMP,
                            accum_out=se[0:mw, :])
                        rse = pkr.tile([P, 1], F32, name="rse")
                        nc.vector.reciprocal(rse[0:mw, :], se[0:mw, :])
                        if store_out:
                            nc.vector.tensor_scalar(
                                out=r_all[0:mw, mb, :], in0=rt[0:mw, :],
                                scalar1=rse[0:mw, :], scalar2=None,
                                op0=mybir.AluOpType.mult)
                            nc.vector.tensor_copy(dist_all[0:mw, mb, :], dps[0:mw, :])
                        else:
                            nc.vector.tensor_scalar(
                                out=rt[0:mw, :], in0=rt[0:mw, :],
                                scalar1=rse[0:mw, :], scalar2=None,
                                op0=mybir.AluOpType.mult)
                            r_tiles.append((rt, realm))
                    return r_tiles

                for it in range(num_iter + 1):
                    st_ari = dram.tile([P, K + 1], F32, name=f"st_ari{it}")
                    st_aro = dram.tile([P, K + 1], F32, addr_space=shr,
                                       name=f"st_aro{it}")
                    r_tiles = dist_pass(muT, store_out=False)
                    cm_ps = pk_stat.tile([P, K], F32, name="cm_ps")
                    cr_ps = pk_stat.tile([1, K], F32, name="cr_ps")
                    for mb in range(NMB):
                        rt, realm = r_tiles[mb]
                        nc.tensor.matmul(
                            cm_ps[:], data_sb[0:realm, mb, :], rt[0:realm, :],
                            start=(mb == 0), stop=(mb == NMB - 1))
                        nc.tensor.matmul(
                            cr_ps[:], ones_col[0:realm, :], rt[0:realm, :],
                            start=(mb == 0), stop=(mb == NMB - 1))
                    # pack stats [128, 51]: cols 0..49 = cmT, col 50 rows 0..49 = cr
                    stats = pk.tile([P, K + 1], F32, name="stats")
                    nc.vector.tensor_copy(stats[:, 0:K], cm_ps[:])
                    cr_sb = pk.tile([1, K], F32, name="cr_sb")
                    nc.vector.tensor_copy(cr_sb[:], cr_ps[:])
                    crT_ps = pk_misc.tile([K, 1], F32, name="crT_ps")
                    nc.tensor.transpose(crT_ps[:], cr_sb[:], ident[0:1, 0:1])
                    nc.vector.tensor_copy(stats[0:K, K:K + 1], crT_ps[:])
                    nc.sync.dma_start(st_ari[:], stats[:])
                    _collective(nc, "AllReduce", mybir.AluOpType.add,
                                st_ari, st_aro, rg, model_single)
                    cm2 = pk.tile([P, K + 1], F32, name="cm2")
                    nc.sync.dma_start(cm2[:], st_aro[:])
                    criT = pk.tile([K, 1], F32, name="criT")
                    nc.vector.reciprocal(criT[:], cm2[0:K, K:K + 1])
                    cri_ps = pk_misc.tile([1, K], F32, name="cri_ps")
                    nc.tensor.transpose(cri_ps[:], criT[:], ident[0:K, 0:K])
                    cri = pk.tile([1, K], F32, name="cri")
                    nc.vector.tensor_copy(cri[:], cri_ps[:])
                    bc_ps = pk_misc.tile([P, K], F32, name="bc_ps")
                    nc.tensor.matmul(bc_ps[:], ones_row[:], cri[:], start=True, stop=True)
                    muT = kmp.tile([P, K], F32, name="muT")
                    nc.vector.tensor_tensor(
                        out=muT[:], in0=cm2[:, 0:K], in1=bc_ps[:],
                        op=mybir.AluOpType.mult)

                # final pass: dist, r outputs with final mu (batched DMAs)
                dist_pass(muT, store_out=True)
                nc.sync.dma_start(
                    or_d[0:(NMB - 1) * P, :].rearrange("(t p) k -> p t k", p=P),
                    r_all[:, 0:NMB - 1, :])
                nc.sync.dma_start(or_d[(NMB - 1) * P:M_LOC, :],
                                  r_all[0:_mb_real(NMB - 1), NMB - 1, :])
                nc.sync.dma_start(
                    odist_d[0:(NMB - 1) * P, :].rearrange("(t p) k -> p t k", p=P),
                    dist_all[:, 0:NMB - 1, :])
                nc.sync.dma_start(odist_d[(NMB - 1) * P:M_LOC, :],
                                  dist_all[0:_mb_real(NMB - 1), NMB - 1, :])

                # mu output: transpose muT -> [K, NOUT]
                mu_st = pk_misc.tile([K, P], F32, name="mu_st")
                nc.tensor.transpose(mu_st[:], muT[:, 0:K], ident[:])
                mu_sb = pk.tile([K, P], F32, name="mu_sb")
                nc.vector.tensor_copy(mu_sb[:], mu_st[:])
                nc.sync.dma_start(omu_d, mu_sb[:])

            if phases != "ABCD":
                with tc.tile_pool(name="zf", bufs=1) as zf:
                    zt = zf.tile([P, max(NOUT, K)], F32, name="zt")
                    nc.vector.memset(zt[:], 0.0)
                    if "D" not in phases:
                        nc.sync.dma_start(omu_d, zt[0:K, 0:NOUT])
                        for mb in range(NMB):
                            realm = _mb_real(mb)
                            nc.sync.dma_start(
                                or_d[mb * P:mb * P + realm, :], zt[0:realm, 0:K])
                            nc.sync.dma_start(
                                odist_d[mb * P:mb * P + realm, :], zt[0:realm, 0:K])
                    if "C" not in phases:
                        for mb in range(NMB):
                            realm = _mb_real(mb)
                            nc.sync.dma_start(
                                oemb_d[mb * P:mb * P + realm, :], zt[0:realm, 0:NOUT])

    nc.compile()
    return nc


_NC_CACHE = {}


def get_nc(num_iter: int):
    if num_iter not in _NC_CACHE:
        _NC_CACHE[num_iter] = build_kernel(num_iter)
    return _NC_CACHE[num_iter]


def kernel(x, adj, W1, b1, W2, b2, init_mu, num_iter):
    x = np.ascontiguousarray(np.asarray(x, dtype=np.float32))
    adj = np.ascontiguousarray(np.asarray(adj, dtype=np.float32))
    W1 = np.ascontiguousarray(np.asarray(W1, dtype=np.float32))
    W2 = np.ascontiguousarray(np.asarray(W2, dtype=np.float32))
    b1 = np.ascontiguousarray(np.asarray(b1, dtype=np.float32))
    b2 = np.ascontiguousarray(np.asarray(b2, dtype=np.float32))
    init_mu = np.ascontiguousarray(np.asarray(init_mu, dtype=np.float32))
    n_it = int(np.asarray(num_iter))

    nc = get_nc(n_it)
    in_maps = []
    for c in range(NC):
        sl = slice(c * M_LOC, (c + 1) * M_LOC)
        in_maps.append({
            "x_sh": x[sl], "adj_sh": adj[sl],
            "W1": W1, "W2": W2, "b1": b1, "b2": b2, "init_mu": init_mu,
        })
    res = run_bass_kernel_spmd(nc, in_maps, core_ids=list(range(NC)))
    outs = res.results
    mu = outs[0]["out_mu"]
    r = np.concatenate([outs[c]["out_r"] for c in range(NC)], axis=0)
    embeds = np.concatenate([outs[c]["out_embeds"] for c in range(NC)], axis=0)
    dist = np.concatenate([outs[c]["out_dist"] for c in range(NC)], axis=0)
    return (mu, r, embeds, dist)


# revision 13
# speedup vs baseline: 1.0117x; 1.0087x over previous
"""GCNClusterNet Trainium2 kernel — 8-core SPMD.

Strategy (hardcoded for N=10000, NFEAT=1024, NHID=512, NOUT=128, K=50):
  - Row-shard x and adj across 8 cores (1250 rows each).
  - Cast adj and x shards to bf16 in DRAM once (SWDGE cast DMA), then feed
    all transposed operands via hardware DMA-transpose loads (few, large).
  - GCN matmuls in bf16 (fp32 PSUM accumulate); bf16 all-gather of xW1 /
    hW2 activations between stages. h kept transposed (hT) end-to-end:
    mm1 emits hT directly so bias+relu+bf16-cast fuse into one ACT op.
  - k-means in fp32: per-iteration partial cluster stats packed [128, 51]
    (cluster_r in column 50) + one AllReduce; softmax without max-sub.
  - Outputs: mu (replicated), r / embeds / dist row-sharded, gathered on host.
"""
import numpy as np

import concourse.bass as bass
import concourse.mybir as mybir
import concourse.tile as tile
import concourse.bacc as bacc
from concourse.bass_utils import run_bass_kernel_spmd
from concourse.masks import make_identity

F32 = mybir.dt.float32
BF16 = mybir.dt.bfloat16
AF = mybir.ActivationFunctionType

NC = 8
N, NFEAT, NHID, NOUT, K = 10000, 1024, 512, 128, 50
TEMP = 30.0
P = 128
M_LOC = N // NC            # 1250 rows per core
NJT = (N + P - 1) // P     # 79 j-tiles (last has 16 valid rows)
JPAD = NJT * P             # 10112
M_PADDED = 1264            # 1250 padded to mult of 16
NMB = (M_LOC + P - 1) // P  # 10 m-blocks of 128 (last 98 valid)
NCT = NHID // P            # 4 c-tiles
NFT = NFEAT // P           # 8 f-tiles
# phase B psum groups: (row offset, m-chunk widths); 4 c-tiles x chunks <= 8 banks
B_GROUPS = [(0, [512, 512]), (1024, [240])]
C_CHUNKS = [512, 512, 240]


def _mb_real(mb):
    return min(P, M_LOC - mb * P)  # valid rows in m-block (98 for mb=9)


def _collective(nc, kind, op, in_ap, out_ap, rg, model, shard_rows=None):
    if not model:
        nc.gpsimd.collective_compute(kind, op, ins=[in_ap.opt()], outs=[out_ap.opt()],
                                     replica_groups=rg)
    elif kind == "AllGather":
        for c in range(NC):
            nc.sync.dma_start(out_ap[c * shard_rows:(c + 1) * shard_rows, :], in_ap[:])
    else:
        nc.sync.dma_start(out_ap[:], in_ap[:])


def build_kernel(num_iter: int, phases: str = "ABCD", model_single: bool = False):
    nc = bacc.Bacc("TRN2", target_bir_lowering=False, debug=False,
                   num_devices=(1 if model_single else NC))

    x_d = nc.dram_tensor("x_sh", [M_LOC, NFEAT], F32, kind="ExternalInput").ap()
    adj_d = nc.dram_tensor("adj_sh", [M_LOC, N], F32, kind="ExternalInput").ap()
    W1_d = nc.dram_tensor("W1", [NFEAT, NHID], F32, kind="ExternalInput").ap()
    W2_d = nc.dram_tensor("W2", [NHID, NOUT], F32, kind="ExternalInput").ap()
    b1_d = nc.dram_tensor("b1", [NHID], F32, kind="ExternalInput").ap()
    b2_d = nc.dram_tensor("b2", [NOUT], F32, kind="ExternalInput").ap()
    mu0_d = nc.dram_tensor("init_mu", [K, NOUT], F32, kind="ExternalInput").ap()

    omu_d = nc.dram_tensor("out_mu", [K, NOUT], F32, kind="ExternalOutput").ap()
    or_d = nc.dram_tensor("out_r", [M_LOC, K], F32, kind="ExternalOutput").ap()
    oemb_d = nc.dram_tensor("out_embeds", [M_LOC, NOUT], F32, kind="ExternalOutput").ap()
    odist_d = nc.dram_tensor("out_dist", [M_LOC, K], F32, kind="ExternalOutput").ap()

    rg = [list(range(NC))]
    shr = "Local" if model_single else "Shared"
    NFB = (N // P) * P  # 9984 full-tile rows of gathered activations

    with tile.TileContext(nc) as tc:
        with (
            tc.tile_pool(name="persist", bufs=1) as pp,
            tc.tile_pool(name="km_state", bufs=2) as kmp,
            tc.tile_pool(name="dram", bufs=1, space="DRAM") as dram,
        ):
            # ---- DRAM scratch ----
            adjb = dram.tile([M_PADDED, JPAD], BF16, name="adjb")
            xb = dram.tile([M_PADDED, NFEAT], BF16, name="xb")
            xw1_agi = dram.tile([M_LOC, NHID], BF16, name="xw1_agi")
            xw1_ago = dram.tile([N, NHID], BF16, addr_space=shr, name="xw1_ago")
            hw2_agi = dram.tile([M_LOC, NOUT], BF16, name="hw2_agi")
            hw2_ago = dram.tile([N, NOUT], BF16, addr_space=shr, name="hw2_ago")

            # ---- persistent SBUF ----
            ones_row = pp.tile([1, P], F32)
            nc.vector.memset(ones_row[:], 1.0)
            ones_col = pp.tile([P, 1], F32)
            nc.vector.memset(ones_col[:], 1.0)
            ident = pp.tile([P, P], F32)
            make_identity(nc, ident[:])
            eps_col = pp.tile([P, 1], F32)
            nc.vector.memset(eps_col[:], 1e-30)
            b1_cols = pp.tile([P, NCT], F32)      # b1 as per-partition columns
            b2_col = pp.tile([P, 1], F32)
            W2_bf = pp.tile([P, NCT, NOUT], BF16)
            hT_sb = pp.tile([P, NCT, M_PADDED], BF16)       # [c, ct, m]
            hw2_sb = pp.tile([P, NJT, NOUT], BF16)          # [j, jt, o]
            dataT_sb = pp.tile([P, M_PADDED], F32)          # [d, m]
            data_sb = pp.tile([P, NMB, P], F32)             # [m, mb, d]
            emb_all = pp.tile([P, NMB, NOUT], F32)          # [m, mb, o]

            # =========== Phase A: casts, weights, x@W1, AG ===========
            if "A" in phases:
             with (
                tc.tile_pool(name="pa_sbuf", bufs=2) as pa,
                tc.tile_pool(name="pa_w", bufs=1) as paw,
                tc.tile_pool(name="pa_psum", bufs=2, space="PSUM") as pa_ps,
             ):
                # adj / x f32 -> bf16 DRAM (SWDGE cast); zero adj pad rows
                nc.gpsimd.dma_start(adjb[0:1024, 0:N], adj_d[0:1024, :])
                nc.gpsimd.dma_start(adjb[1024:M_LOC, 0:N], adj_d[1024:M_LOC, :])
                zpad = pa.tile([16, JPAD], BF16, name="zpad")
                nc.vector.memset(zpad[:], 0.0)
                nc.sync.dma_start(adjb[M_LOC:M_PADDED, :], zpad[0:M_PADDED - M_LOC, :])
                nc.gpsimd.dma_start(xb[0:M_LOC, :], x_d)

                # weights / biases
                W1_bf = paw.tile([P, NFT, NHID], BF16)
                nc.gpsimd.dma_start(W1_bf[:], W1_d.rearrange("(t p) c -> p t c", p=P))
                nc.gpsimd.dma_start(W2_bf[:], W2_d.rearrange("(t p) c -> p t c", p=P))
                nc.sync.dma_start(b1_cols[:], b1_d.rearrange("(t p) -> p t", p=P))
                nc.sync.dma_start(b2_col[:], b2_d.rearrange("(p a) -> p a", a=1))

                # xT via DMA-transpose: 8 loads of [M_PADDED, 128] -> [128, M_PADDED]
                xT_sb = paw.tile([P, NFT, M_PADDED], BF16)
                for ft in range(NFT):
                    nc.sync.dma_start_transpose(
                        xT_sb[:, ft, :], xb[:, ft * P:(ft + 1) * P])

                # xW1 = x @ W1 (bf16, natural out), write AG input
                for mb in range(NMB):
                    realm = _mb_real(mb)
                    mw = min(P, M_PADDED - mb * P)
                    acc = pa_ps.tile([P, NHID], F32, name="xw1_acc")
                    for ft in range(NFT):
                        nc.tensor.matmul(
                            acc[0:mw, :],
                            xT_sb[:, ft, mb * P:mb * P + mw],
                            W1_bf[:, ft, :],
                            start=(ft == 0), stop=(ft == NFT - 1))
                    xw1_bf = pa.tile([P, NHID], BF16, name="xw1_bf")
                    nc.vector.tensor_copy(xw1_bf[0:realm, :], acc[0:realm, :])
                    nc.sync.dma_start(
                        xw1_agi[mb * P:mb * P + realm, :], xw1_bf[0:realm, :])

                _collective(nc, "AllGather", mybir.AluOpType.bypass,
                            xw1_agi, xw1_ago, rg, model_single, M_LOC)

            # ====== Phase B: hT = relu(adj @ xW1 + b1)^T  (direct hT out) ======
            if "B" in phases:
             with (
                tc.tile_pool(name="pb_xw1", bufs=1) as pbx,
                tc.tile_pool(name="pb_sbuf", bufs=3) as pb,
                tc.tile_pool(name="pb_acc", bufs=1, space="PSUM") as pb_acc,
             ):
                xw1_sb = pbx.tile([P, NJT, NHID], BF16)  # [j, jt, c]
                nc.sync.dma_start(
                    xw1_sb[:, 0:N // P, :],
                    xw1_ago[0:NFB, :].rearrange("(t p) c -> p t c", p=P))
                nc.sync.dma_start(xw1_sb[0:N - NFB, N // P, :], xw1_ago[NFB:N, :])

                for gi, (g0, chunks) in enumerate(B_GROUPS):
                    grows = sum(chunks)
                    accs = {}
                    for ct in range(NCT):
                        for mi, cw in enumerate(chunks):
                            accs[(ct, mi)] = pb_acc.tile(
                                [P, cw], F32, name=f"hT_acc{ct}_{mi}")
                    for jt in range(NJT):
                        kj = min(P, N - jt * P)
                        adjT = pb.tile([P, 1024], BF16, name="adjT_b")
                        nc.sync.dma_start_transpose(
                            adjT[:, 0:grows],
                            adjb[g0:g0 + grows, jt * P:(jt + 1) * P])
                        for ct in range(NCT):
                            for mi, cw in enumerate(chunks):
                                moff = sum(chunks[:mi])
                                nc.tensor.matmul(
                                    accs[(ct, mi)][:],
                                    xw1_sb[0:kj, jt, ct * P:(ct + 1) * P],
                                    adjT[0:kj, moff:moff + cw],
                                    start=(jt == 0), stop=(jt == NJT - 1))
                    for ct in range(NCT):
                        for mi, cw in enumerate(chunks):
                            moff = g0 + sum(chunks[:mi])
                            nc.scalar.activation(
                                hT_sb[:, ct, moff:moff + cw], accs[(ct, mi)][:],
                                AF.Relu, bias=b1_cols[:, ct:ct + 1])

            # =========== Phase B2: hW2 = h @ W2 (bf16) + AG ===========
            if "B" in phases and "C" in phases:
             with (
                tc.tile_pool(name="pc_sbuf", bufs=2) as pc,
                tc.tile_pool(name="pc_psum", bufs=2, space="PSUM") as pc_ps,
             ):
                for mb in range(NMB):
                    realm = _mb_real(mb)
                    mw = min(P, M_PADDED - mb * P)
                    acc = pc_ps.tile([P, NOUT], F32, name="hw2_acc")
                    for ct in range(NCT):
                        nc.tensor.matmul(
                            acc[0:mw, :],
                            hT_sb[:, ct, mb * P:mb * P + mw],
                            W2_bf[:, ct, :],
                            start=(ct == 0), stop=(ct == NCT - 1))
                    hw2_bf = pc.tile([P, NOUT], BF16, name="hw2_bf")
                    nc.vector.tensor_copy(hw2_bf[0:realm, :], acc[0:realm, :])
                    nc.sync.dma_start(
                        hw2_agi[mb * P:mb * P + realm, :], hw2_bf[0:realm, :])
                _collective(nc, "AllGather", mybir.AluOpType.bypass,
                            hw2_agi, hw2_ago, rg, model_single, M_LOC)
                nc.sync.dma_start(
                    hw2_sb[:, 0:N // P, :],
                    hw2_ago[0:NFB, :].rearrange("(t p) c -> p t c", p=P))
                nc.sync.dma_start(hw2_sb[0:N - NFB, N // P, :], hw2_ago[NFB:N, :])

            # ==== Phase C: embedsT = (adj @ hW2 + b2)^T (j-outer); data ====
            if "C" in phases:
             with (
                tc.tile_pool(name="pd_sbuf", bufs=3) as pd,
                tc.tile_pool(name="pd_eps", bufs=2) as pde,
                tc.tile_pool(name="pd_acc", bufs=1, space="PSUM") as pd_acc,
                tc.tile_pool(name="pd_stage", bufs=2, space="PSUM") as pd_st,
             ):
                eT_sb = pd.tile([P, M_PADDED], F32, bufs=1, name="eT_sb")  # [o, m]
                eaccs = [pd_acc.tile([P, cw], F32, name=f"eT_acc{mi}")
                         for mi, cw in enumerate(C_CHUNKS)]
                for jt in range(NJT):
                    kj = min(P, N - jt * P)
                    adjT = pd.tile([P, M_PADDED], BF16, name="adjT_c", bufs=8)
                    nc.sync.dma_start_transpose(
                        adjT[:], adjb[:, jt * P:(jt + 1) * P])
                    for mi, cw in enumerate(C_CHUNKS):
                        moff = sum(C_CHUNKS[:mi])
                        nc.tensor.matmul(
                            eaccs[mi][:],
                            hw2_sb[0:kj, jt, :],
                            adjT[0:kj, moff:moff + cw],
                            start=(jt == 0), stop=(jt == NJT - 1))
                for mi, cw in enumerate(C_CHUNKS):
                    moff = sum(C_CHUNKS[:mi])
                    nc.vector.tensor_scalar(
                        out=eT_sb[:, moff:moff + cw], in0=eaccs[mi][:],
                        scalar1=b2_col[:], scalar2=None, op0=mybir.AluOpType.add)

                # per m-block: embeds natural, norms, data, dataT
                for mb in range(NMB):
                    realm = _mb_real(mb)
                    mw = min(P, M_PADDED - mb * P)
                    st = pd_st.tile([P, P], F32, name="e_st")
                    nc.tensor.transpose(
                        st[0:mw, :], eT_sb[:, mb * P:mb * P + mw], ident[:])
                    nc.vector.tensor_copy(emb_all[0:mw, mb, :], st[0:mw, :])
                    sq = pde.tile([P, NOUT], F32, name="sq")
                    n2 = pde.tile([P, 1], F32, name="n2")
                    nc.scalar.activation(
                        sq[0:mw, :], emb_all[0:mw, mb, :], AF.Square,
                        accum_out=n2[0:mw, :])
                    nrm = pde.tile([P, 1], F32, name="nrm")
                    nc.scalar.activation(
                        nrm[0:mw, :], n2[0:mw, :], AF.Sqrt, bias=eps_col[0:mw, :])
                    rnrm = pde.tile([P, 1], F32, name="rnrm")
                    nc.vector.reciprocal(rnrm[0:mw, :], nrm[0:mw, :])
                    nc.vector.tensor_scalar(
                        out=data_sb[0:mw, mb, :], in0=emb_all[0:mw, mb, :],
                        scalar1=rnrm[0:mw, :], scalar2=None, op0=mybir.AluOpType.mult)
                    st2 = pd_st.tile([P, P], F32, name="d_st")
                    nc.tensor.transpose(
                        st2[:, 0:mw], data_sb[0:mw, mb, :], ident[0:mw, 0:mw])
                    nc.vector.tensor_copy(dataT_sb[:, mb * P:mb * P + mw], st2[:, 0:mw])
                # embeds out (batched: 9 full tiles + 98-row tail)
                nc.sync.dma_start(
                    oemb_d[0:(NMB - 1) * P, :].rearrange("(t p) c -> p t c", p=P),
                    emb_all[:, 0:NMB - 1, :])
                nc.sync.dma_start(oemb_d[(NMB - 1) * P:M_LOC, :],
                                  emb_all[0:_mb_real(NMB - 1), NMB - 1, :])

            # =========== Phase D: k-means ===========
            if "D" in phases:
             with (
                tc.tile_pool(name="pk_sbuf", bufs=2) as pk,
                tc.tile_pool(name="pk_r", bufs=12) as pkr,
                tc.tile_pool(name="pk_dist", bufs=2, space="PSUM") as pk_dist,
                tc.tile_pool(name="pk_stat", bufs=1, space="PSUM") as pk_stat,
                tc.tile_pool(name="pk_misc", bufs=1, space="PSUM") as pk_misc,
             ):
                muT = kmp.tile([P, K], F32, name="muT0")
                nc.sync.dma_start(muT[:], mu0_d.rearrange("k d -> d k"))

                r_all = pp.tile([P, NMB, K], F32)
                dist_all = pp.tile([P, NMB, K], F32)

                def dist_pass(muT_cur, store_out):
                    dps = pk_dist.tile([P, NMB * K], F32, name="dist_ps")
                    for mb in range(NMB):
                        mw = min(P, M_PADDED - mb * P)
                        nc.tensor.matmul(
                            dps[0:mw, mb * K:(mb + 1) * K],
                            dataT_sb[:, mb * P:mb * P + mw],
                            muT_cur[:], start=True, stop=True)
                    ex = pkr.tile([P, NMB, K], F32, name="ex")
                    nc.scalar.activation(
                        ex[:].rearrange("p a b -> p (a b)"), dps[:],
                        AF.Exp, scale=TEMP)
                    se = pkr.tile([P, NMB], F32, name="se")
                    nc.vector.reduce_sum(
                        se[:].unsqueeze(2), ex[:],
                        axis=mybir.AxisListType.X)
                    rse = pkr.tile([P, NMB], F32, name="rse")
                    nc.vector.reciprocal(rse[:], se[:])
                    nc.vector.tensor_tensor(
                        out=r_all[:], in0=ex[:],
                        in1=rse[:].unsqueeze(2).broadcast_to([P, NMB, K]),
                        op=mybir.AluOpType.mult)
                    if store_out:
                        nc.vector.tensor_copy(
                            dist_all[:].rearrange("p a b -> p (a b)"), dps[:])

                for it in range(num_iter + 1):
                    st_ari = dram.tile([P, K + 1], F32, name=f"st_ari{it}")
                    st_aro = dram.tile([P, K + 1], F32, addr_space=shr,
                                       name=f"st_aro{it}")
                    dist_pass(muT, store_out=False)
                    cm_ps = pk_stat.tile([P, K], F32, name="cm_ps")
                    cr_ps = pk_stat.tile([1, K], F32, name="cr_ps")
                    for mb in range(NMB):
                        realm = _mb_real(mb)
                        nc.tensor.matmul(
                            cm_ps[:], data_sb[0:realm, mb, :], r_all[0:realm, mb, :],
                            start=(mb == 0), stop=(mb == NMB - 1))
                        nc.tensor.matmul(
                            cr_ps[:], ones_col[0:realm, :], r_all[0:realm, mb, :],
                            start=(mb == 0), stop=(mb == NMB - 1))
                    # pack stats [128, 51]: cols 0..49 = cmT, col 50 rows 0..49 = cr
                    stats = pk.tile([P, K + 1], F32, name="stats")
                    nc.vector.tensor_copy(stats[:, 0:K], cm_ps[:])
                    cr_sb = pk.tile([1, K], F32, name="cr_sb")
                    nc.vector.tensor_copy(cr_sb[:], cr_ps[:])
                    crT_ps = pk_misc.tile([K, 1], F32, name="crT_ps")
                    nc.tensor.transpose(crT_ps[:], cr_sb[:], ident[0:1, 0:1])
                    nc.vector.tensor_copy(stats[0:K, K:K + 1], crT_ps[:])
                    nc.sync.dma_start(st_ari[:], stats[:])
                    _collective(nc, "AllReduce", mybir.AluOpType.add,
                                st_ari, st_aro, rg, model_single)
                    cm2 = pk.tile([P, K + 1], F32, name="cm2")
                    nc.sync.dma_start(cm2[:], st_aro[:])
                    criT = pk.tile([K, 1], F32, name="criT")
                    nc.vector.reciprocal(criT[:], cm2[0:K, K:K + 1])
                    cri_ps = pk_misc.tile([1, K], F32, name="cri_ps")
                    nc.tensor.transpose(cri_ps[:], criT[:], ident[0:K, 0:K])
                    cri = pk.tile([1, K], F32, name="cri")
                    nc.vector.tensor_copy(cri[:], cri_ps[:])
                    bc_ps = pk_misc.tile([P, K], F32, name="bc_ps")
                    nc.tensor.matmul(bc_ps[:], ones_row[:], cri[:], start=True, stop=True)
                    muT = kmp.tile([P, K], F32, name="muT")
                    nc.vector.tensor_tensor(
                        out=muT[:], in0=cm2[:, 0:K], in1=bc_ps[:],
                        op=mybir.AluOpType.mult)

                # final pass: dist, r outputs with final mu (batched DMAs)
                dist_pass(muT, store_out=True)
                nc.sync.dma_start(
                    or_d[0:(NMB - 1) * P, :].rearrange("(t p) k -> p t k", p=P),
                    r_all[:, 0:NMB - 1, :])
                nc.sync.dma_start(or_d[(NMB - 1) * P:M_LOC, :],
                                  r_all[0:_mb_real(NMB - 1), NMB - 1, :])
                nc.sync.dma_start(
                    odist_d[0:(NMB - 1) * P, :].rearrange("(t p) k -> p t k", p=P),
                    dist_all[:, 0:NMB - 1, :])
                nc.sync.dma_start(odist_d[(NMB - 1) * P:M_LOC, :],
                                  dist_all[0:_mb_real(NMB - 1), NMB - 1, :])

                # mu output: transpose muT -> [K, NOUT]
                mu_st = pk_misc.tile([K, P], F32, name="mu_st")
                nc.tensor.transpose(mu_st[:], muT[:, 0:K], ident[:])
                mu_sb = pk.tile([K, P], F32, name="mu_sb")
                nc.vector.tensor_copy(mu_sb[:], mu_st[:])
                nc.sync.dma_start(omu_d, mu_sb[:])

            if phases != "ABCD":
                with tc.tile_pool(name="zf", bufs=1) as zf:
                    zt = zf.tile([P, max(NOUT, K)], F32, name="zt")
                    nc.vector.memset(zt[:], 0.0)
                    if "D" not in phases:
                        nc.sync.dma_start(omu_d, zt[0:K, 0:NOUT])
                        for mb in range(NMB):
                            realm = _mb_real(mb)
                            nc.sync.dma_start(
                                or_d[mb * P:mb * P + realm, :], zt[0:realm, 0:K])
                            nc.sync.dma_start(
                                odist_d[mb * P:mb * P + realm, :], zt[0:realm, 0:K])
                    if "C" not in phases:
                        for mb in range(NMB):
                            realm = _mb_real(mb)
                            nc.sync.dma_start(
                                oemb_d[mb * P:mb * P + realm, :], zt[0:realm, 0:NOUT])

    nc.compile()
    return nc


_NC_CACHE = {}


def get_nc(num_iter: int):
    if num_iter not in _NC_CACHE:
        _NC_CACHE[num_iter] = build_kernel(num_iter)
    return _NC_CACHE[num_iter]


def kernel(x, adj, W1, b1, W2, b2, init_mu, num_iter):
    x = np.ascontiguousarray(np.asarray(x, dtype=np.float32))
    adj = np.ascontiguousarray(np.asarray(adj, dtype=np.float32))
    W1 = np.ascontiguousarray(np.asarray(W1, dtype=np.float32))
    W2 = np.ascontiguousarray(np.asarray(W2, dtype=np.float32))
    b1 = np.ascontiguousarray(np.asarray(b1, dtype=np.float32))
    b2 = np.ascontiguousarray(np.asarray(b2, dtype=np.float32))
    init_mu = np.ascontiguousarray(np.asarray(init_mu, dtype=np.float32))
    n_it = int(np.asarray(num_iter))

    nc = get_nc(n_it)
    in_maps = []
    for c in range(NC):
        sl = slice(c * M_LOC, (c + 1) * M_LOC)
        in_maps.append({
            "x_sh": x[sl], "adj_sh": adj[sl],
            "W1": W1, "W2": W2, "b1": b1, "b2": b2, "init_mu": init_mu,
        })
    res = run_bass_kernel_spmd(nc, in_maps, core_ids=list(range(NC)))
    outs = res.results
    mu = outs[0]["out_mu"]
    r = np.concatenate([outs[c]["out_r"] for c in range(NC)], axis=0)
    embeds = np.concatenate([outs[c]["out_embeds"] for c in range(NC)], axis=0)
    dist = np.concatenate([outs[c]["out_dist"] for c in range(NC)], axis=0)
    return (mu, r, embeds, dist)


# revision 15
# speedup vs baseline: 1.0759x; 1.0635x over previous
"""GCNClusterNet Trainium2 kernel — 8-core SPMD.

Strategy (hardcoded for N=10000, NFEAT=1024, NHID=512, NOUT=128, K=50):
  - Row-shard x and adj across 8 cores (1250 rows each).
  - Cast adj and x shards to bf16 in DRAM once (SWDGE cast DMA), then feed
    all transposed operands via hardware DMA-transpose loads (few, large).
  - GCN matmuls in bf16 (fp32 PSUM accumulate); bf16 all-gather of xW1 /
    hW2 activations between stages. h kept transposed (hT) end-to-end:
    mm1 emits hT directly so bias+relu+bf16-cast fuse into one ACT op.
  - k-means in fp32: per-iteration partial cluster stats packed [128, 51]
    (cluster_r in column 50) + one AllReduce; softmax without max-sub.
  - Outputs: mu (replicated), r / embeds / dist row-sharded, gathered on host.
"""
import numpy as np

import concourse.bass as bass
import concourse.mybir as mybir
import concourse.tile as tile
import concourse.bacc as bacc
from concourse.bass_utils import run_bass_kernel_spmd
from concourse.masks import make_identity

F32 = mybir.dt.float32
BF16 = mybir.dt.bfloat16
AF = mybir.ActivationFunctionType

NC = 8
N, NFEAT, NHID, NOUT, K = 10000, 1024, 512, 128, 50
TEMP = 30.0
P = 128
M_LOC = N // NC            # 1250 rows per core
NJT = (N + P - 1) // P     # 79 j-tiles (last has 16 valid rows)
JPAD = NJT * P             # 10112
M_PADDED = 1264            # 1250 padded to mult of 16
NMB = (M_LOC + P - 1) // P  # 10 m-blocks of 128 (last 98 valid)
NCT = NHID // P            # 4 c-tiles
NFT = NFEAT // P           # 8 f-tiles
# phase B psum groups: (row offset, m-chunk widths); 4 c-tiles x chunks <= 8 banks
B_GROUPS = [(0, [512, 512]), (1024, [240])]
C_CHUNKS = [512, 512, 240]


def _mb_real(mb):
    return min(P, M_LOC - mb * P)  # valid rows in m-block (98 for mb=9)


def _collective(nc, kind, op, in_ap, out_ap, rg, model, shard_rows=None):
    if not model:
        nc.gpsimd.collective_compute(kind, op, ins=[in_ap.opt()], outs=[out_ap.opt()],
                                     replica_groups=rg)
    elif kind == "AllGather":
        # timing proxy: one shard copy (real AG latency is close to this)
        nc.sync.dma_start(out_ap[0:shard_rows, :], in_ap[:])
    else:
        nc.sync.dma_start(out_ap[:], in_ap[:])


def build_kernel(num_iter: int, phases: str = "ABCD", model_single: bool = False):
    nc = bacc.Bacc("TRN2", target_bir_lowering=False, debug=False,
                   num_devices=(1 if model_single else NC))

    x_d = nc.dram_tensor("x_sh", [M_LOC, NFEAT], F32, kind="ExternalInput").ap()
    adj_d = nc.dram_tensor("adj_sh", [M_LOC, N], F32, kind="ExternalInput").ap()
    W1_d = nc.dram_tensor("W1", [NFEAT, NHID], F32, kind="ExternalInput").ap()
    W2_d = nc.dram_tensor("W2", [NHID, NOUT], F32, kind="ExternalInput").ap()
    b1_d = nc.dram_tensor("b1", [NHID], F32, kind="ExternalInput").ap()
    b2_d = nc.dram_tensor("b2", [NOUT], F32, kind="ExternalInput").ap()
    mu0_d = nc.dram_tensor("init_mu", [K, NOUT], F32, kind="ExternalInput").ap()

    omu_d = nc.dram_tensor("out_mu", [K, NOUT], F32, kind="ExternalOutput").ap()
    or_d = nc.dram_tensor("out_r", [M_LOC, K], F32, kind="ExternalOutput").ap()
    oemb_d = nc.dram_tensor("out_embeds", [M_LOC, NOUT], F32, kind="ExternalOutput").ap()
    odist_d = nc.dram_tensor("out_dist", [M_LOC, K], F32, kind="ExternalOutput").ap()

    rg = [list(range(NC))]
    shr = "Local" if model_single else "Shared"
    NFB = (N // P) * P  # 9984 full-tile rows of gathered activations

    with tile.TileContext(nc) as tc:
        with (
            tc.tile_pool(name="persist", bufs=1) as pp,
            tc.tile_pool(name="km_state", bufs=2) as kmp,
            tc.tile_pool(name="dram", bufs=1, space="DRAM") as dram,
        ):
            # ---- DRAM scratch ----
            adjb = dram.tile([M_PADDED, JPAD], BF16, name="adjb")
            xb = dram.tile([M_PADDED, NFEAT], BF16, name="xb")
            xw1_agi = dram.tile([M_LOC, NHID], BF16, name="xw1_agi")
            xw1_ago = dram.tile([N, NHID], BF16, addr_space=shr, name="xw1_ago")
            hw2_agi = dram.tile([M_LOC, NOUT], BF16, name="hw2_agi")
            hw2_ago = dram.tile([N, NOUT], BF16, addr_space=shr, name="hw2_ago")

            # ---- persistent SBUF ----
            ones_row = pp.tile([1, P], F32)
            nc.vector.memset(ones_row[:], 1.0)
            ones_col = pp.tile([P, 1], F32)
            nc.vector.memset(ones_col[:], 1.0)
            ident = pp.tile([P, P], F32)
            make_identity(nc, ident[:])
            eps_col = pp.tile([P, 1], F32)
            nc.vector.memset(eps_col[:], 1e-30)
            b1_cols = pp.tile([P, NCT], F32)      # b1 as per-partition columns
            b2_col = pp.tile([P, 1], F32)
            W2_bf = pp.tile([P, NCT, NOUT], BF16)
            hT_sb = pp.tile([P, NCT, M_PADDED], BF16)       # [c, ct, m]
            hw2_sb = pp.tile([P, NJT, NOUT], BF16)          # [j, jt, o]
            dataT_sb = pp.tile([P, M_PADDED], F32)          # [d, m]
            data_sb = pp.tile([P, NMB, P], F32)             # [m, mb, d]
            emb_all = pp.tile([P, NMB, NOUT], F32)          # [m, mb, o]

            # =========== Phase A: casts, weights, x@W1, AG ===========
            if "A" in phases:
             with (
                tc.tile_pool(name="pa_sbuf", bufs=2) as pa,
                tc.tile_pool(name="pa_w", bufs=1) as paw,
                tc.tile_pool(name="pa_psum", bufs=2, space="PSUM") as pa_ps,
             ):
                # adj / x f32 -> bf16 DRAM (SWDGE cast); zero adj pad rows
                nc.gpsimd.dma_start(adjb[0:1024, 0:N], adj_d[0:1024, :])
                nc.gpsimd.dma_start(adjb[1024:M_LOC, 0:N], adj_d[1024:M_LOC, :])
                zpad = pa.tile([16, JPAD], BF16, name="zpad")
                nc.vector.memset(zpad[:], 0.0)
                nc.sync.dma_start(adjb[M_LOC:M_PADDED, :], zpad[0:M_PADDED - M_LOC, :])
                nc.gpsimd.dma_start(xb[0:M_LOC, :], x_d)

                # weights / biases
                W1_bf = paw.tile([P, NFT, NHID], BF16)
                nc.gpsimd.dma_start(W1_bf[:], W1_d.rearrange("(t p) c -> p t c", p=P))
                nc.gpsimd.dma_start(W2_bf[:], W2_d.rearrange("(t p) c -> p t c", p=P))
                nc.sync.dma_start(b1_cols[:], b1_d.rearrange("(t p) -> p t", p=P))
                nc.sync.dma_start(b2_col[:], b2_d.rearrange("(p a) -> p a", a=1))

                # xT via DMA-transpose: 8 loads of [M_PADDED, 128] -> [128, M_PADDED]
                xT_sb = paw.tile([P, NFT, M_PADDED], BF16)
                for ft in range(NFT):
                    nc.sync.dma_start_transpose(
                        xT_sb[:, ft, :], xb[:, ft * P:(ft + 1) * P])

                # xW1 = x @ W1 (bf16, natural out), write AG input
                for mb in range(NMB):
                    realm = _mb_real(mb)
                    mw = min(P, M_PADDED - mb * P)
                    acc = pa_ps.tile([P, NHID], F32, name="xw1_acc")
                    for ft in range(NFT):
                        nc.tensor.matmul(
                            acc[0:mw, :],
                            xT_sb[:, ft, mb * P:mb * P + mw],
                            W1_bf[:, ft, :],
                            start=(ft == 0), stop=(ft == NFT - 1))
                    xw1_bf = pa.tile([P, NHID], BF16, name="xw1_bf")
                    nc.vector.tensor_copy(xw1_bf[0:realm, :], acc[0:realm, :])
                    nc.sync.dma_start(
                        xw1_agi[mb * P:mb * P + realm, :], xw1_bf[0:realm, :])

                _collective(nc, "AllGather", mybir.AluOpType.bypass,
                            xw1_agi, xw1_ago, rg, model_single, M_LOC)

            # ====== Phase B: hT = relu(adj @ xW1 + b1)^T  (direct hT out) ======
            if "B" in phases:
             with (
                tc.tile_pool(name="pb_xw1", bufs=1) as pbx,
                tc.tile_pool(name="pb_sbuf", bufs=3) as pb,
                tc.tile_pool(name="pb_acc", bufs=1, space="PSUM") as pb_acc,
             ):
                xw1_sb = pbx.tile([P, NJT, NHID], BF16)  # [j, jt, c]
                nc.sync.dma_start(
                    xw1_sb[:, 0:N // P, :],
                    xw1_ago[0:NFB, :].rearrange("(t p) c -> p t c", p=P))
                nc.sync.dma_start(xw1_sb[0:N - NFB, N // P, :], xw1_ago[NFB:N, :])

                for gi, (g0, chunks) in enumerate(B_GROUPS):
                    grows = sum(chunks)
                    accs = {}
                    for ct in range(NCT):
                        for mi, cw in enumerate(chunks):
                            accs[(ct, mi)] = pb_acc.tile(
                                [P, cw], F32, name=f"hT_acc{ct}_{mi}")
                    for jt in range(NJT):
                        kj = min(P, N - jt * P)
                        adjT = pb.tile([P, 1024], BF16, name="adjT_b")
                        nc.sync.dma_start_transpose(
                            adjT[:, 0:grows],
                            adjb[g0:g0 + grows, jt * P:(jt + 1) * P])
                        for ct in range(NCT):
                            for mi, cw in enumerate(chunks):
                                moff = sum(chunks[:mi])
                                nc.tensor.matmul(
                                    accs[(ct, mi)][:],
                                    xw1_sb[0:kj, jt, ct * P:(ct + 1) * P],
                                    adjT[0:kj, moff:moff + cw],
                                    start=(jt == 0), stop=(jt == NJT - 1))
                    for ct in range(NCT):
                        for mi, cw in enumerate(chunks):
                            moff = g0 + sum(chunks[:mi])
                            nc.scalar.activation(
                                hT_sb[:, ct, moff:moff + cw], accs[(ct, mi)][:],
                                AF.Relu, bias=b1_cols[:, ct:ct + 1])

            # =========== Phase B2: hW2 = h @ W2 (bf16) + AG ===========
            if "B" in phases and "C" in phases:
             with (
                tc.tile_pool(name="pc_sbuf", bufs=2) as pc,
                tc.tile_pool(name="pc_psum", bufs=2, space="PSUM") as pc_ps,
             ):
                for mb in range(NMB):
                    realm = _mb_real(mb)
                    mw = min(P, M_PADDED - mb * P)
                    acc = pc_ps.tile([P, NOUT], F32, name="hw2_acc")
                    for ct in range(NCT):
                        nc.tensor.matmul(
                            acc[0:mw, :],
                            hT_sb[:, ct, mb * P:mb * P + mw],
                            W2_bf[:, ct, :],
                            start=(ct == 0), stop=(ct == NCT - 1))
                    hw2_bf = pc.tile([P, NOUT], BF16, name="hw2_bf")
                    nc.vector.tensor_copy(hw2_bf[0:realm, :], acc[0:realm, :])
                    nc.sync.dma_start(
                        hw2_agi[mb * P:mb * P + realm, :], hw2_bf[0:realm, :])
                _collective(nc, "AllGather", mybir.AluOpType.bypass,
                            hw2_agi, hw2_ago, rg, model_single, M_LOC)
                nc.sync.dma_start(
                    hw2_sb[:, 0:N // P, :],
                    hw2_ago[0:NFB, :].rearrange("(t p) c -> p t c", p=P))
                nc.sync.dma_start(hw2_sb[0:N - NFB, N // P, :], hw2_ago[NFB:N, :])

            # ==== Phase C: embedsT = (adj @ hW2 + b2)^T (j-outer); data ====
            if "C" in phases:
             with (
                tc.tile_pool(name="pd_sbuf", bufs=3) as pd,
                tc.tile_pool(name="pd_eps", bufs=2) as pde,
                tc.tile_pool(name="pd_acc", bufs=1, space="PSUM") as pd_acc,
                tc.tile_pool(name="pd_stage", bufs=2, space="PSUM") as pd_st,
             ):
                eT_sb = pd.tile([P, M_PADDED], F32, bufs=1, name="eT_sb")  # [o, m]
                eaccs = [pd_acc.tile([P, cw], F32, name=f"eT_acc{mi}")
                         for mi, cw in enumerate(C_CHUNKS)]
                for jt in range(NJT):
                    kj = min(P, N - jt * P)
                    adjT = pd.tile([P, M_PADDED], BF16, name="adjT_c", bufs=8)
                    nc.sync.dma_start_transpose(
                        adjT[:], adjb[:, jt * P:(jt + 1) * P])
                    for mi, cw in enumerate(C_CHUNKS):
                        moff = sum(C_CHUNKS[:mi])
                        nc.tensor.matmul(
                            eaccs[mi][:],
                            hw2_sb[0:kj, jt, :],
                            adjT[0:kj, moff:moff + cw],
                            start=(jt == 0), stop=(jt == NJT - 1))
                for mi, cw in enumerate(C_CHUNKS):
                    moff = sum(C_CHUNKS[:mi])
                    nc.vector.tensor_scalar(
                        out=eT_sb[:, moff:moff + cw], in0=eaccs[mi][:],
                        scalar1=b2_col[:], scalar2=None, op0=mybir.AluOpType.add)

                # per m-block: embeds natural, norms, data, dataT
                for mb in range(NMB):
                    realm = _mb_real(mb)
                    mw = min(P, M_PADDED - mb * P)
                    st = pd_st.tile([P, P], F32, name="e_st")
                    nc.tensor.transpose(
                        st[0:mw, :], eT_sb[:, mb * P:mb * P + mw], ident[:])
                    nc.vector.tensor_copy(emb_all[0:mw, mb, :], st[0:mw, :])
                    sq = pde.tile([P, NOUT], F32, name="sq")
                    n2 = pde.tile([P, 1], F32, name="n2")
                    nc.scalar.activation(
                        sq[0:mw, :], emb_all[0:mw, mb, :], AF.Square,
                        accum_out=n2[0:mw, :])
                    nrm = pde.tile([P, 1], F32, name="nrm")
                    nc.scalar.activation(
                        nrm[0:mw, :], n2[0:mw, :], AF.Sqrt, bias=eps_col[0:mw, :])
                    rnrm = pde.tile([P, 1], F32, name="rnrm")
                    nc.vector.reciprocal(rnrm[0:mw, :], nrm[0:mw, :])
                    nc.vector.tensor_scalar(
                        out=data_sb[0:mw, mb, :], in0=emb_all[0:mw, mb, :],
                        scalar1=rnrm[0:mw, :], scalar2=None, op0=mybir.AluOpType.mult)
                    st2 = pd_st.tile([P, P], F32, name="d_st")
                    nc.tensor.transpose(
                        st2[:, 0:mw], data_sb[0:mw, mb, :], ident[0:mw, 0:mw])
                    nc.vector.tensor_copy(dataT_sb[:, mb * P:mb * P + mw], st2[:, 0:mw])
                # embeds out (batched: 9 full tiles + 98-row tail)
                nc.sync.dma_start(
                    oemb_d[0:(NMB - 1) * P, :].rearrange("(t p) c -> p t c", p=P),
                    emb_all[:, 0:NMB - 1, :])
                nc.sync.dma_start(oemb_d[(NMB - 1) * P:M_LOC, :],
                                  emb_all[0:_mb_real(NMB - 1), NMB - 1, :])

            # =========== Phase D: k-means ===========
            if "D" in phases:
             with (
                tc.tile_pool(name="pk_sbuf", bufs=2) as pk,
                tc.tile_pool(name="pk_r", bufs=12) as pkr,
                tc.tile_pool(name="pk_dist", bufs=2, space="PSUM") as pk_dist,
                tc.tile_pool(name="pk_stat", bufs=1, space="PSUM") as pk_stat,
                tc.tile_pool(name="pk_misc", bufs=1, space="PSUM") as pk_misc,
             ):
                muT = kmp.tile([P, K], F32, name="muT0")
                nc.sync.dma_start(muT[:], mu0_d.rearrange("k d -> d k"))

                r_all = pp.tile([P, NMB, K], F32)
                dist_all = pp.tile([P, NMB, K], F32)

                def dist_pass(muT_cur, store_out):
                    dps = pk_dist.tile([P, NMB * K], F32, name="dist_ps")
                    for mb in range(NMB):
                        mw = min(P, M_PADDED - mb * P)
                        nc.tensor.matmul(
                            dps[0:mw, mb * K:(mb + 1) * K],
                            dataT_sb[:, mb * P:mb * P + mw],
                            muT_cur[:], start=True, stop=True)
                    ex = pkr.tile([P, NMB, K], F32, name="ex")
                    nc.scalar.activation(
                        ex[:].rearrange("p a b -> p (a b)"), dps[:],
                        AF.Exp, scale=TEMP)
                    se = pkr.tile([P, NMB], F32, name="se")
                    nc.vector.reduce_sum(
                        se[:].unsqueeze(2), ex[:],
                        axis=mybir.AxisListType.X)
                    rse = pkr.tile([P, NMB], F32, name="rse")
                    nc.vector.reciprocal(rse[:], se[:])
                    nc.vector.tensor_tensor(
                        out=r_all[:], in0=ex[:],
                        in1=rse[:].unsqueeze(2).broadcast_to([P, NMB, K]),
                        op=mybir.AluOpType.mult)
                    if store_out:
                        nc.vector.tensor_copy(
                            dist_all[:].rearrange("p a b -> p (a b)"), dps[:])

                for it in range(num_iter + 1):
                    st_ari = dram.tile([P, K + 1], F32, name=f"st_ari{it}")
                    st_aro = dram.tile([P * NC, K + 1], F32, addr_space=shr,
                                       name=f"st_aro{it}")
                    dist_pass(muT, store_out=False)
                    cm_ps = pk_stat.tile([P, K], F32, name="cm_ps")
                    cr_ps = pk_stat.tile([1, K], F32, name="cr_ps")
                    for mb in range(NMB):
                        realm = _mb_real(mb)
                        nc.tensor.matmul(
                            cm_ps[:], data_sb[0:realm, mb, :], r_all[0:realm, mb, :],
                            start=(mb == 0), stop=(mb == NMB - 1))
                        nc.tensor.matmul(
                            cr_ps[:], ones_col[0:realm, :], r_all[0:realm, mb, :],
                            start=(mb == 0), stop=(mb == NMB - 1))
                    # pack stats [128, 51]: cols 0..49 = cmT, col 50 rows 0..49 = cr
                    stats = pk.tile([P, K + 1], F32, name="stats")
                    nc.vector.tensor_copy(stats[:, 0:K], cm_ps[:])
                    cr_sb = pk.tile([1, K], F32, name="cr_sb")
                    nc.vector.tensor_copy(cr_sb[:], cr_ps[:])
                    crT_ps = pk_misc.tile([K, 1], F32, name="crT_ps")
                    nc.tensor.transpose(crT_ps[:], cr_sb[:], ident[0:1, 0:1])
                    nc.vector.tensor_copy(stats[0:K, K:K + 1], crT_ps[:])
                    nc.sync.dma_start(st_ari[:], stats[:])
                    _collective(nc, "AllGather", mybir.AluOpType.bypass,
                                st_ari, st_aro, rg, model_single, P)
                    all8 = pk.tile([P, NC, K + 1], F32, name="all8")
                    nc.sync.dma_start(
                        all8[:], st_aro[:].rearrange("(t p) k -> p t k", p=P))
                    cm2 = pk.tile([P, K + 1], F32, name="cm2")
                    nc.vector.reduce_sum(
                        cm2[:].unsqueeze(2),
                        all8[:].rearrange("p t k -> p k t"),
                        axis=mybir.AxisListType.X)
                    criT = pk.tile([K, 1], F32, name="criT")
                    nc.vector.reciprocal(criT[:], cm2[0:K, K:K + 1])
                    cri_ps = pk_misc.tile([1, K], F32, name="cri_ps")
                    nc.tensor.transpose(cri_ps[:], criT[:], ident[0:K, 0:K])
                    cri = pk.tile([1, K], F32, name="cri")
                    nc.vector.tensor_copy(cri[:], cri_ps[:])
                    bc_ps = pk_misc.tile([P, K], F32, name="bc_ps")
                    nc.tensor.matmul(bc_ps[:], ones_row[:], cri[:], start=True, stop=True)
                    muT = kmp.tile([P, K], F32, name="muT")
                    nc.vector.tensor_tensor(
                        out=muT[:], in0=cm2[:, 0:K], in1=bc_ps[:],
                        op=mybir.AluOpType.mult)

                # final pass: dist, r outputs with final mu (batched DMAs)
                dist_pass(muT, store_out=True)
                nc.sync.dma_start(
                    or_d[0:(NMB - 1) * P, :].rearrange("(t p) k -> p t k", p=P),
                    r_all[:, 0:NMB - 1, :])
                nc.sync.dma_start(or_d[(NMB - 1) * P:M_LOC, :],
                                  r_all[0:_mb_real(NMB - 1), NMB - 1, :])
                nc.sync.dma_start(
                    odist_d[0:(NMB - 1) * P, :].rearrange("(t p) k -> p t k", p=P),
                    dist_all[:, 0:NMB - 1, :])
                nc.sync.dma_start(odist_d[(NMB - 1) * P:M_LOC, :],
                                  dist_all[0:_mb_real(NMB - 1), NMB - 1, :])

                # mu output: transpose muT -> [K, NOUT]
                mu_st = pk_misc.tile([K, P], F32, name="mu_st")
                nc.tensor.transpose(mu_st[:], muT[:, 0:K], ident[:])
                mu_sb = pk.tile([K, P], F32, name="mu_sb")
                nc.vector.tensor_copy(mu_sb[:], mu_st[:])
                nc.sync.dma_start(omu_d, mu_sb[:])

            if phases != "ABCD":
                with tc.tile_pool(name="zf", bufs=1) as zf:
                    zt = zf.tile([P, max(NOUT, K)], F32, name="zt")
                    nc.vector.memset(zt[:], 0.0)
                    if "D" not in phases:
                        nc.sync.dma_start(omu_d, zt[0:K, 0:NOUT])
                        for mb in range(NMB):
                            realm = _mb_real(mb)
                            nc.sync.dma_start(
                                or_d[mb * P:mb * P + realm, :], zt[0:realm, 0:K])
                            nc.sync.dma_start(
                                odist_d[mb * P:mb * P + realm, :], zt[0:realm, 0:K])
                    if "C" not in phases:
                        for mb in range(NMB):
                            realm = _mb_real(mb)
                            nc.sync.dma_start(
                                oemb_d[mb * P:mb * P + realm, :], zt[0:realm, 0:NOUT])

    nc.compile()
    return nc


_NC_CACHE = {}


def get_nc(num_iter: int):
    if num_iter not in _NC_CACHE:
        _NC_CACHE[num_iter] = build_kernel(num_iter)
    return _NC_CACHE[num_iter]


def kernel(x, adj, W1, b1, W2, b2, init_mu, num_iter):
    x = np.ascontiguousarray(np.asarray(x, dtype=np.float32))
    adj = np.ascontiguousarray(np.asarray(adj, dtype=np.float32))
    W1 = np.ascontiguousarray(np.asarray(W1, dtype=np.float32))
    W2 = np.ascontiguousarray(np.asarray(W2, dtype=np.float32))
    b1 = np.ascontiguousarray(np.asarray(b1, dtype=np.float32))
    b2 = np.ascontiguousarray(np.asarray(b2, dtype=np.float32))
    init_mu = np.ascontiguousarray(np.asarray(init_mu, dtype=np.float32))
    n_it = int(np.asarray(num_iter))

    nc = get_nc(n_it)
    in_maps = []
    for c in range(NC):
        sl = slice(c * M_LOC, (c + 1) * M_LOC)
        in_maps.append({
            "x_sh": x[sl], "adj_sh": adj[sl],
            "W1": W1, "W2": W2, "b1": b1, "b2": b2, "init_mu": init_mu,
        })
    res = run_bass_kernel_spmd(nc, in_maps, core_ids=list(range(NC)))
    outs = res.results
    mu = outs[0]["out_mu"]
    r = np.concatenate([outs[c]["out_r"] for c in range(NC)], axis=0)
    embeds = np.concatenate([outs[c]["out_embeds"] for c in range(NC)], axis=0)
    dist = np.concatenate([outs[c]["out_dist"] for c in range(NC)], axis=0)
    return (mu, r, embeds, dist)
